# revision 1
# baseline (speedup 1.0000x reference)
"""Trainium2 Bass kernel for nn_ConvGraph_SC (gnn_message_passing).

Reference computation (per batch b of 64, N=32 nodes, C=512 channels, 7x7 spatial):
    state = input.mean(axis=(3,4))                       # [B, N, C]
    mat1  = state @ W1.T + b1
    mat2  = state @ W2.T + b2
    adj   = mat1 @ mat2.T                                # [B, N, N]
    soft  = softmax((adj - mean(adj)) / std(adj), rows)  # global mean/std, ddof=1
    out   = mean(soft @ state + state, axis=1)           # [B, C]

Device-side algebra (S = spatial SUM of x, unscaled):
  * softmax((adj-mu)/sigma) is invariant to a per-batch affine rescale of adj,
    so we work with adj' = 2401*adj = S A S^T + 49*s_u 1^T + 49*1 s_v^T + c0'
    where A = W1^T W2 (UNSCALED -> fp16-friendly magnitudes), u' = 49 W1^T b2,
    v' = 49 W2^T b1, c0' = 2401 b1.b2.
  * Row-constant terms (s_u, c0) drop out of the row softmax; they enter only
    the global mean/std, computed from per-row sums with closed-form
    corrections. No row-max subtraction before exp: after global mean/std
    normalization the argument is bounded (|z| ~< 10), safe in f32.
  * fp16 datapath on the PE (1 cycle/row vs 4 for fp32): state sums, A, u, v
    in fp16; PSUM accumulation stays fp32. rel-err budget is 2e-2; this lands
    ~5e-4.
  * The 52us of spatial-sum reduction is DVE-bound (tensor_reduce is 1
    elem/cycle for every dtype; only fp16 tensor_tensor gets 2x). Three
    halves run a self-contained f32 pairwise-add tree on the otherwise-idle
    GpSimd; the rest reduce directly on DVE.
  * Epilogue out[c] = sum_k (colsum(soft)[k]+1)/(N*HW) * S[k,c] runs on the
    PE: stationary wf4[p,r] = w[p//4]*(p%4==r) against moving sraw -> [4,128]
    natural output.
  * Stats groups are [4, 3, 1]: group 1's serial chain overlaps batch 7's
    stream; only batch 7's one-batch chain remains as the tail. Batch 7
    itself streams as 4 quarter-DMAs reduced on arrival.

Sharding: pure data parallel, 8 batches per NeuronCore, weights replicated.
"""

import numpy as np

import concourse.bacc as bacc
import concourse.tile as tile
from concourse import masks, mybir
from concourse.bass_utils import run_bass_kernel_spmd

F32 = mybir.dt.float32
F16 = mybir.dt.float16
I32 = mybir.dt.int32
NCORES = 8
B, N, C, HW = 64, 32, 512, 49
BPC = B // NCORES          # batches per core
FREE = N * C * HW // 128   # 6272 floats per partition per batch
HALF = FREE // 2           # 3136
QUAR = FREE // 4           # 1568
K1023 = float(np.sqrt(np.float64(1023.0)))
# stats groups [4, 3, 1]: the tail after the last input byte is only batch
# 7's single-batch chain
GSIZE = [4, 3, 1]
GBASE = [0, 4, 7]
GRP_OF_B = [0, 0, 0, 0, 1, 1, 1, 2]
BP_OF_B = [0, 1, 2, 3, 0, 1, 2, 0]
# halves (i = 2b + h) in GPS run an f32 tensor_tensor pairwise tree on the
# otherwise-idle GpSimd engine; the rest reduce directly on DVE
GPS = {2, 3, 8}

_CACHED_NC = None

A_ = mybir.AluOpType


def build_bass():
    nc = bacc.Bacc("TRN2", target_bir_lowering=False)

    x_d = nc.declare_dram_parameter("x", [BPC, 128, FREE], F32, isOutput=False)
    a_d = nc.declare_dram_parameter("amat", [C, C], F16, isOutput=False)
    uv_d = nc.declare_dram_parameter("uv", [C, 2], F16, isOutput=False)
    c0_d = nc.declare_dram_parameter("c0", [32, 1], F32, isOutput=False)
    msk_d = nc.declare_dram_parameter("msk", [128, 36], F16, isOutput=False)
    out_d = nc.declare_dram_parameter("out", [4, 128 * BPC], F32, isOutput=True)

    lp = nc.allow_low_precision("fp16 spatial sums; rel-err budget 2e-2")
    lp.__enter__()
    with tile.TileContext(nc) as tc:
        with (
            tc.tile_pool(name="xpool", bufs=8) as xpool,
            tc.tile_pool(name="xqpool", bufs=4) as xqpool,
            tc.tile_pool(name="tgpool", bufs=2) as tgpool,
            tc.tile_pool(name="singles", bufs=1) as singles,
            tc.tile_pool(name="tap", bufs=2) as tap,
            tc.tile_pool(name="small", bufs=2) as small,
            tc.tile_pool(name="ps_t", bufs=1, space="PSUM") as ps_t_pool,
            tc.tile_pool(name="ps_tt", bufs=2, space="PSUM") as ps_tt_pool,
            tc.tile_pool(name="ps_adj", bufs=2, space="PSUM") as ps_adj_pool,
            tc.tile_pool(name="ps_misc", bufs=2, space="PSUM") as ps_misc_pool,
            tc.tile_pool(name="ps_out", bufs=1, space="PSUM") as ps_out_pool,
        ):
            # ---- persistent tiles -----------------------------------------
            ident = singles.tile([128, 128], F16)
            ones16 = singles.tile([1, 128], F16)
            ones_col = singles.tile([32, 1], F32)
            ones_r32 = singles.tile([1, 32], F32)
            a_sb = singles.tile([128, 4 * C], F16)
            uv_sb = singles.tile([128, 8], F16)
            c0_sb = singles.tile([32, 1], F32)
            msk_sb = singles.tile([128, 36], F16)  # [:, :32]=M32, [:, 32:36]=mask4
            # sraw: spatial sums, natural layout: [p=(4n+c_hi), c_low], fp16
            sraw_all = singles.tile([128, 128 * BPC], F16)
            # state^T: [p=c_low, 128b + 4n + c_hi], fp16
            st_all = singles.tile([128, 128 * BPC], F16)
            outsb = singles.tile([4, 128 * BPC], F32)

            def load_weights():
                # emitted after the first batch's x DMAs so the input stream
                # owns the head of the DMA queues
                for r in range(4):
                    nc.sync.dma_start(
                        out=a_sb[:, 512 * r : 512 * (r + 1)],
                        in_=a_d[128 * r : 128 * (r + 1), :],
                    )
                for r in range(4):
                    nc.sync.dma_start(
                        out=uv_sb[:, 2 * r : 2 * (r + 1)],
                        in_=uv_d[128 * r : 128 * (r + 1), :],
                    )
                nc.sync.dma_start(out=c0_sb[:], in_=c0_d[:])
                nc.sync.dma_start(out=msk_sb[:], in_=msk_d[:])
                masks.make_identity(nc, ident[:])
                nc.gpsimd.memset(ones16[:], 1.0)
                nc.gpsimd.memset(ones_col[:], 1.0)
                nc.gpsimd.memset(ones_r32[:], 1.0)

            # per-group state (allocated lazily in program order)
            grp = {}

            def start_group(g):
                gs = GSIZE[g]
                grp[g] = {
                    # adj [:32, 0:128]; sv rows [0:1, 128+32bp : 160+32bp]
                    "ps_adj": ps_adj_pool.tile([32, 256], F32, name="ps_adj"),
                    # ps_misc regions: su cols [:32, 0:4], stats colsum
                    # [:1, 8:16], stats bcast [:32, 16:24], wf colsums
                    # [:1, 64+32bp], wfb bcast [:, 192+32bp]
                    "ps_misc": ps_misc_pool.tile([128, 512], F32, name="ps_misc"),
                    "ps_out": ps_out_pool.tile([4, 128 * gs], F32, name="ps_out"),
                    "sv": small.tile([1, 32 * gs], F16, tag="sv", name="sv"),
                    "q": small.tile([32, gs], F32, tag="q", name="q"),
                    "t": small.tile([32, gs], F32, tag="t", name="t"),
                    "rq": small.tile([32, gs], F32, tag="rq", name="rq"),
                    "expt": small.tile([32, 32 * gs], F32, tag="expt", name="expt"),
                }

            def tree_reduce(eng, dst, src, scratch):
                # dst[p, q] = sum_s src[p, q, s], s=49, via pairwise adds
                tv = scratch[:].rearrange("p (q s) -> p q s", s=24)
                eng.tensor_add(tv[:, :, 0:24], src[:, :, 0:24], src[:, :, 25:49])
                eng.tensor_add(tv[:, :, 0:12], tv[:, :, 0:12], tv[:, :, 12:24])
                eng.tensor_add(tv[:, :, 0:6], tv[:, :, 0:6], tv[:, :, 6:12])
                eng.tensor_add(tv[:, :, 0:3], tv[:, :, 0:3], tv[:, :, 3:6])
                d3 = dst.rearrange("p (q s) -> p q s", s=1)
                eng.tensor_add(d3[:], tv[:, :, 0:1], tv[:, :, 1:2])
                eng.tensor_add(d3[:], d3[:], tv[:, :, 2:3])
                eng.tensor_add(d3[:], d3[:], src[:, :, 24:25])

            def per_batch(b):
                g, bp = GRP_OF_B[b], BP_OF_B[b]
                if bp == 0:
                    start_group(g)
                gd = grp[g]
                scol = slice(128 * b, 128 * (b + 1))

                # -- load + spatial sum (fp16) --------------------------------
                if b < BPC - 1:
                    for h in range(2):
                        i = 2 * b + h
                        xb = xpool.tile([128, HALF], F32, tag="xb")
                        nc.sync.dma_start(
                            out=xb[:], in_=x_d[b, :, HALF * h : HALF * (h + 1)]
                        )
                        dst = sraw_all[:, 128 * b + 64 * h : 128 * b + 64 * (h + 1)]
                        xv = xb[:].rearrange("p (q s) -> p q s", s=HW)
                        if i in GPS:
                            tg = tgpool.tile([128, 1536], F32, tag="tg")
                            tree_reduce(nc.gpsimd, dst, xv, tg)
                        else:
                            nc.vector.reduce_sum(
                                out=dst, in_=xv, axis=mybir.AxisListType.X,
                            )
                    if b == 0:
                        load_weights()
                    # transpose sraw -> st (fp16, via PE)
                    ps_t = ps_t_pool.tile([128, 128], F16)
                    nc.tensor.transpose(ps_t[:], sraw_all[:, scol], ident[:])
                    nc.scalar.copy(st_all[:, scol], ps_t[:])
                else:
                    # last batch: quarters, direct-reduced, half-transposes
                    ps_t = ps_t_pool.tile([128, 128], F16)
                    for qi in range(4):
                        xb = xqpool.tile([128, QUAR], F32, tag="xbq")
                        nc.sync.dma_start(
                            out=xb[:], in_=x_d[b, :, QUAR * qi : QUAR * (qi + 1)]
                        )
                        nc.vector.reduce_sum(
                            out=sraw_all[
                                :, 128 * b + 32 * qi : 128 * b + 32 * (qi + 1)
                            ],
                            in_=xb[:].rearrange("p (q s) -> p q s", s=HW),
                            axis=mybir.AxisListType.X,
                        )
                        if qi % 2 == 1:
                            hh = qi // 2
                            nc.tensor.transpose(
                                ps_t[64 * hh : 64 * (hh + 1), :],
                                sraw_all[
                                    :, 128 * b + 64 * hh : 128 * b + 64 * (hh + 1)
                                ],
                                ident[:],
                            )
                    nc.scalar.copy(st_all[:, scol], ps_t[:])

                def st_slice(r):
                    return st_all[:, 128 * b + r : 128 * (b + 1) : 4]

                # -- TA^T = A^T S^T directly: [p=d_low, 32s+k] ---------------
                ps_tt = ps_tt_pool.tile([128, 128], F32)
                for s in range(4):
                    for r in range(4):
                        nc.tensor.matmul(
                            ps_tt[:, 32 * s : 32 * (s + 1)],
                            a_sb[:, 512 * r + 128 * s : 512 * r + 128 * (s + 1)],
                            st_slice(r),
                            start=(r == 0), stop=(r == 3),
                        )
                ta_b = tap.tile([128, 128], F16, tag="ta")
                nc.scalar.copy(ta_b[:], ps_tt[:])

                # -- su column + sv row --------------------------------------
                ps_misc = gd["ps_misc"]
                for r in range(4):
                    nc.tensor.matmul(
                        ps_misc[:32, bp : bp + 1],
                        st_slice(r),
                        uv_sb[:, 2 * r : 2 * r + 1],
                        start=(r == 0), stop=(r == 3),
                    )
                svsl = slice(128 + 32 * bp, 128 + 32 * (bp + 1))
                for r in range(4):
                    nc.tensor.matmul(
                        gd["ps_adj"][0:1, svsl],
                        uv_sb[:, 2 * r + 1 : 2 * r + 2],
                        st_slice(r),
                        start=(r == 0), stop=(r == 3),
                    )
                nc.scalar.copy(
                    gd["sv"][:, 32 * bp : 32 * (bp + 1)], gd["ps_adj"][0:1, svsl]
                )

                # -- adjacency': TA S^T + 1 sv^T (minus row-constants) -------
                ps_adj = gd["ps_adj"]
                asl = slice(32 * bp, 32 * (bp + 1))
                for s in range(4):
                    nc.tensor.matmul(
                        ps_adj[:, asl],
                        ta_b[:, 32 * s : 32 * (s + 1)],
                        st_slice(s),
                        start=(s == 0), stop=False,
                    )
                nc.tensor.matmul(
                    ps_adj[:, asl],
                    ones16[0:1, 0:32],
                    gd["sv"][0:1, 32 * bp : 32 * (bp + 1)],
                    start=False, stop=True,
                )

                # -- per-batch stats pieces ----------------------------------
                nc.vector.reduce_sum(
                    out=gd["t"][:, bp : bp + 1], in_=ps_adj[:, asl],
                    axis=mybir.AxisListType.X,
                )
                sq_scr = small.tile([32, 32], F32, tag="sq_scr")
                nc.scalar.activation(
                    out=sq_scr[:], in_=ps_adj[:, asl],
                    func=mybir.ActivationFunctionType.Square,
                    accum_out=gd["rq"][:, bp : bp + 1],
                )
                nc.scalar.activation(
                    out=gd["q"][:, bp : bp + 1], in_=ps_misc[:32, bp : bp + 1],
                    func=mybir.ActivationFunctionType.Identity,
                    bias=c0_sb[:], scale=1.0,
                )

            def finish_group(g):
                gd = grp[g]
                gs = GSIZE[g]
                ps_adj, ps_misc, ps_out = gd["ps_adj"], gd["ps_misc"], gd["ps_out"]
                q_g, t_g, rowsq = gd["q"], gd["t"], gd["rq"]

                # ---- stats: S1/S2 of TRUE adj' via row sums ----------------
                # stats_g: cols 0:gs = S1 rows, gs:2gs = S2 rows
                stats_g = small.tile([32, 2 * gs], F32, tag="stats_g", name="sg")
                q32 = small.tile([32, gs], F32, tag="q32", name="q32")
                nc.vector.tensor_scalar(
                    out=q32[:], in0=q_g[:], scalar1=32.0, scalar2=None, op0=A_.mult,
                )
                nc.vector.tensor_add(stats_g[:, 0:gs], q32[:], t_g[:])
                # S2row = rowsq + q*(2t + 32q); 2t + 32q = t + S1row
                h_g = small.tile([32, gs], F32, tag="h_g", name="h_g")
                nc.vector.tensor_add(h_g[:], t_g[:], stats_g[:, 0:gs])
                s2c = small.tile([32, gs], F32, tag="s2c", name="s2c")
                nc.vector.tensor_mul(s2c[:], q_g[:], h_g[:])
                nc.vector.tensor_add(stats_g[:, gs : 2 * gs], rowsq[:], s2c[:])

                # cross-partition sum + broadcast back (PE ones trick)
                nc.tensor.matmul(
                    ps_misc[:1, 8 : 8 + 2 * gs], ones_col[:], stats_g[:],
                    start=True, stop=True,
                )
                s_sb = small.tile([1, 2 * gs], F32, tag="s_sb", name="s_sb")
                nc.scalar.copy(s_sb[:], ps_misc[:1, 8 : 8 + 2 * gs])
                nc.tensor.matmul(
                    ps_misc[:32, 16 : 16 + 2 * gs], ones_r32[:], s_sb[:],
                    start=True, stop=True,
                )
                s_all = small.tile([32, 2 * gs], F32, tag="s_all", name="s_all")
                nc.scalar.copy(s_all[:], ps_misc[:32, 16 : 16 + 2 * gs])

                # ---- inv_std = sqrt(1023)/sqrt(S2 - S1^2/1024) -------------
                t1 = small.tile([32, gs], F32, tag="t1", name="t1")
                nc.vector.tensor_mul(t1[:], s_all[:, 0:gs], s_all[:, 0:gs])
                nc.vector.tensor_scalar(
                    out=t1[:], in0=t1[:], scalar1=-1.0 / 1024.0, scalar2=None,
                    op0=A_.mult,
                )
                v1023 = small.tile([32, gs], F32, tag="v1023", name="v1023")
                nc.vector.tensor_add(v1023[:], t1[:], s_all[:, gs : 2 * gs])
                # Newton rsqrt with magic seed, 2 iterations (~4e-6 rel err);
                # exp stays the only activation table the scalar engine needs
                yint = small.tile([32, gs], I32, tag="yint", name="yint")
                nc.vector.tensor_scalar(
                    out=yint[:], in0=v1023[:].bitcast(I32), scalar1=1,
                    scalar2=None, op0=A_.logical_shift_right,
                )
                nc.vector.tensor_scalar(
                    out=yint[:], in0=yint[:], scalar1=-1,
                    scalar2=0x5F3759DF, op0=A_.mult, op1=A_.add,
                )
                y = small.tile([32, gs], F32, tag="y", name="y")
                nc.vector.tensor_copy(y[:], yint[:].bitcast(F32))
                ya = small.tile([32, gs], F32, tag="ya", name="ya")
                yb = small.tile([32, gs], F32, tag="yb", name="yb")
                for it in range(2):
                    nc.vector.tensor_mul(ya[:], y[:], y[:])
                    nc.vector.tensor_mul(yb[:], ya[:], v1023[:])
                    last = it == 1
                    nc.vector.tensor_scalar(
                        out=ya[:], in0=yb[:],
                        scalar1=(-0.5 * K1023) if last else -0.5,
                        scalar2=(1.5 * K1023) if last else 1.5,
                        op0=A_.mult, op1=A_.add,
                    )
                    nc.vector.tensor_mul(y[:], y[:], ya[:])
                inv_g = y  # [32, gs] inv_std per batch column

                # ---- softmax + epilogue ------------------------------------
                # no row-max subtraction: after global mean/std normalization
                # the exp argument is bounded (|z| < ~10), safe in f32; row
                # constants cancel in the row softmax
                expt = gd["expt"]
                for bp in range(gs):
                    nc.scalar.activation(
                        out=expt[:, 32 * bp : 32 * (bp + 1)],
                        in_=ps_adj[:, 32 * bp : 32 * (bp + 1)],
                        func=mybir.ActivationFunctionType.Exp,
                        bias=0.0, scale=inv_g[:, bp : bp + 1],
                    )
                rowsum = small.tile([32, gs], F32, tag="rowsum", name="rs")
                nc.vector.reduce_sum(
                    out=rowsum[:],
                    in_=expt[:, 0 : 32 * gs].rearrange("p (b m) -> p b m", m=32),
                    axis=mybir.AxisListType.X,
                )
                recip = small.tile([32, gs], F32, tag="recip", name="recip")
                nc.vector.reciprocal(recip[:], rowsum[:])

                # w[k] = colsum(soft) per batch -> [1, 32] rows at partition 0
                for bp in range(gs):
                    nc.tensor.matmul(
                        ps_misc[:1, 64 + 32 * bp : 64 + 32 * (bp + 1)],
                        recip[:, bp : bp + 1],
                        expt[:, 32 * bp : 32 * (bp + 1)],
                        start=True, stop=True,
                    )
                wf16 = small.tile([1, 32 * gs], F16, tag="wf16", name="wf16")
                nc.vector.tensor_scalar(
                    out=wf16[:], in0=ps_misc[:1, 64 : 64 + 32 * gs],
                    scalar1=1.0 / (N * HW), scalar2=1.0 / (N * HW),
                    op0=A_.mult, op1=A_.add,
                )
                for bp in range(gs):
                    b = GBASE[g] + bp
                    # broadcast wf to all partitions, then gather the diagonal
                    # wfb[p, p//4] and spread to wf4[p, r] = w[p//4]*(p%4==r)
                    wsl = slice(192 + 32 * bp, 224 + 32 * bp)
                    nc.tensor.matmul(
                        ps_misc[:, wsl],
                        ones16[:],
                        wf16[0:1, 32 * bp : 32 * (bp + 1)],
                        start=True, stop=True,
                    )
                    scr32 = small.tile([128, 32], F32, tag="scr32", name="scr32")
                    dcol = small.tile([128, 1], F32, tag="dcol", name="dcol")
                    nc.vector.scalar_tensor_tensor(
                        out=scr32[:], in0=ps_misc[:, wsl], scalar=1.0,
                        in1=msk_sb[:, 0:32], op0=A_.mult, op1=A_.mult,
                        accum_out=dcol[:],
                    )
                    wf4 = small.tile([128, 4], F16, tag="wf4", name="wf4")
                    nc.gpsimd.tensor_scalar(
                        out=wf4[:], in0=msk_sb[:, 32:36], scalar1=dcol[:],
                        scalar2=None, op0=A_.mult,
                    )
                    nc.tensor.matmul(
                        ps_out[:4, 128 * bp : 128 * (bp + 1)],
                        wf4[:],
                        sraw_all[:, 128 * b : 128 * (b + 1)],
                        start=True, stop=True,
                    )
                    nc.scalar.copy(
                        outsb[:, 128 * b : 128 * (b + 1)],
                        ps_out[:4, 128 * bp : 128 * (bp + 1)],
                    )
                gb = GBASE[g]
                nc.sync.dma_start(
                    out=out_d[:, 128 * gb : 128 * (gb + gs)],
                    in_=outsb[:, 128 * gb : 128 * (gb + gs)],
                )

            # schedule: each group's serial reduction chain is emitted after
            # later batches' stream work so engine queues never head-of-line
            # block the stream; the tail is only batch 7's one-batch chain
            for b in range(6):
                per_batch(b)
            finish_group(0)
            per_batch(6)
            per_batch(7)
            finish_group(1)
            finish_group(2)

    lp.__exit__(None, None, None)
    nc.finalize()
    return nc


def host_prep(input, W1, b1, W2, b2):
    input = np.ascontiguousarray(input, dtype=np.float32)
    w1 = np.asarray(W1, dtype=np.float64)
    w2 = np.asarray(W2, dtype=np.float64)
    b1 = np.asarray(b1, dtype=np.float64)
    b2 = np.asarray(b2, dtype=np.float64)
    # softmax((adj-mu)/sigma) is scale-invariant per batch: use 2401*adj so A
    # stays in fp16-normal range
    amat = np.ascontiguousarray(w1.T @ w2, dtype=np.float16)
    u = HW * (w1.T @ b2)
    v = HW * (w2.T @ b1)
    uv = np.ascontiguousarray(np.stack([u, v], axis=1), dtype=np.float16)
    c0 = np.full((32, 1), float(HW * HW * (b1 @ b2)), dtype=np.float32)
    p = np.arange(128)
    m32 = (np.arange(32)[None, :] == (p[:, None] // 4)).astype(np.float16)
    m4 = (np.arange(4)[None, :] == (p[:, None] % 4)).astype(np.float16)
    msk = np.ascontiguousarray(np.concatenate([m32, m4], axis=1))
    return input, amat, uv, c0, msk


def make_in_maps(input, W1, b1, W2, b2):
    input, amat, uv, c0, msk = host_prep(input, W1, b1, W2, b2)
    in_maps = []
    for i in range(NCORES):
        shard = input[BPC * i : BPC * (i + 1)].reshape(BPC, 128, FREE)
        in_maps.append(
            {"x": shard, "amat": amat, "uv": uv, "c0": c0, "msk": msk}
        )
    return in_maps


def kernel(input, W1, b1, W2, b2):
    global _CACHED_NC
    if _CACHED_NC is None:
        _CACHED_NC = build_bass()
    nc = _CACHED_NC

    in_maps = make_in_maps(input, W1, b1, W2, b2)
    res = run_bass_kernel_spmd(nc, in_maps, list(range(NCORES)))

    out = np.empty((B, C), dtype=np.float32)
    for i in range(NCORES):
        o = res.results[i]["out"]  # [4, 128*BPC], out[b, 128r+q] = o[r, 128b+q]
        out[BPC * i : BPC * (i + 1)] = (
            o.reshape(4, BPC, 128).transpose(1, 0, 2).reshape(BPC, C)
        )
    return out



# revision 2
# speedup vs baseline: 1.1567x; 1.1567x over previous
"""Trainium2 Bass kernel for nn_ConvGraph_SC (gnn_message_passing).

Reference computation (per batch b of 64, N=32 nodes, C=512 channels, 7x7 spatial):
    state = input.mean(axis=(3,4))                       # [B, N, C]
    mat1  = state @ W1.T + b1
    mat2  = state @ W2.T + b2
    adj   = mat1 @ mat2.T                                # [B, N, N]
    soft  = softmax((adj - mean(adj)) / std(adj), rows)  # global mean/std, ddof=1
    out   = mean(soft @ state + state, axis=1)           # [B, C]

Device-side algebra (S = spatial SUM of x, unscaled):
  * softmax((adj-mu)/sigma) is invariant to a per-batch affine rescale of adj,
    so we work with adj' = 2401*adj = S A S^T + 49*s_u 1^T + 49*1 s_v^T + c0'
    where A = W1^T W2, u' = 49 W1^T b2, v' = 49 W2^T b1, c0' = 2401 b1.b2.
  * Row-constant terms (s_u, c0) drop out of the row softmax; they enter only
    the global mean/std, computed from per-row sums with closed-form
    corrections. No row-max subtraction before exp: after global mean/std
    normalization the argument is bounded (|z| ~< 10), safe in f32.
  * x is shipped fp16 (rel-err budget is 2e-2; fp16 rounding of the spatial
    sum lands ~5e-4) -> HBM traffic halves to ~13.1 MB/core + 0.54 MB weights.
  * Per-partition row layout is [s(49), c_low(128)] so the spatial-sum
    pairwise-add tree works on contiguous, 4B-aligned fp16 runs -> DVE
    tensor_tensor 2x mode (2 elem/cycle).  GpSimd runs batch 0's tree to
    keep DVE under the DMA stream rate.
  * All 8 batch buffers stay SBUF-resident (~100 KB/partition) so the input
    stream never stalls on buffer recycling.
  * Per-batch adj row sums / row sums-of-squares run on the Scalar engine
    (activation accum_out), keeping DVE free for the trees.
  * Batch 7 streams as tapered chunks (24/16/8/1 spatial slabs) reduced on
    arrival, so the post-last-byte tail is short.
  * Epilogue out[c] = sum_k (colsum(soft)[k]+1)/(N*HW) * S[k,c] runs on the
    PE: stationary wf4[p,r] = w[p//4]*(p%4==r) against moving sraw.

Sharding: pure data parallel, 8 batches per NeuronCore, weights replicated.
"""

import numpy as np

import concourse.bacc as bacc
import concourse.tile as tile
from concourse import masks, mybir
from concourse.bass_utils import run_bass_kernel_spmd

F32 = mybir.dt.float32
F16 = mybir.dt.float16
I32 = mybir.dt.int32
NCORES = 8
B, N, C, HW = 64, 32, 512, 49
BPC = B // NCORES          # batches per core
FREE = N * C * HW // 128   # 6272 fp16 elems per partition per batch
SLAB = 128                 # one spatial position = 128 c_low elems
K1023 = float(np.sqrt(np.float64(1023.0)))
# stats groups [4, 3, 1]: the tail after the last input byte is only batch
# 7's single-batch chain
GSIZE = [4, 3, 1]
GBASE = [0, 4, 7]
GRP_OF_B = [0, 0, 0, 0, 1, 1, 1, 2]
BP_OF_B = [0, 1, 2, 3, 0, 1, 2, 0]
# batch 7 tapered chunks, in spatial slabs
CHUNKS = [24, 16, 8, 1]
# weights blob columns (fp16): amat 0:2048, uv 2048:2056, c0 2056, msk 2058:2094
WCOLS = 2096

_CACHED_NC = None

A_ = mybir.AluOpType


def build_bass():
    nc = bacc.Bacc("TRN2", target_bir_lowering=False)

    x_d = nc.declare_dram_parameter("x", [BPC, 128, FREE], F16, isOutput=False)
    w_d = nc.declare_dram_parameter("wblob", [128, WCOLS], F16, isOutput=False)
    out_d = nc.declare_dram_parameter("out", [4, 128 * BPC], F32, isOutput=True)

    lp = nc.allow_low_precision("fp16 input + spatial sums; rel-err budget 2e-2")
    lp.__enter__()
    with tile.TileContext(nc) as tc:
        with (
            tc.tile_pool(name="tgpool", bufs=2) as tgpool,
            tc.tile_pool(name="singles", bufs=1) as singles,
            tc.tile_pool(name="tap", bufs=2) as tap,
            tc.tile_pool(name="small", bufs=2) as small,
            tc.tile_pool(name="ps_t", bufs=1, space="PSUM") as ps_t_pool,
            tc.tile_pool(name="ps_tt", bufs=2, space="PSUM") as ps_tt_pool,
            tc.tile_pool(name="ps_adj", bufs=2, space="PSUM") as ps_adj_pool,
            tc.tile_pool(name="ps_misc", bufs=2, space="PSUM") as ps_misc_pool,
            tc.tile_pool(name="ps_out", bufs=1, space="PSUM") as ps_out_pool,
        ):
            # ---- persistent tiles -----------------------------------------
            x_all = singles.tile([128, FREE * BPC], F16)
            ident = singles.tile([128, 128], F16)
            ones16 = singles.tile([1, 128], F16)
            ones_col = singles.tile([32, 1], F32)
            ones_r32 = singles.tile([1, 32], F32)
            wsb = singles.tile([128, WCOLS], F16)
            c0_sb = singles.tile([32, 1], F32)
            gscr = singles.tile([128, 24 * SLAB], F16)   # gpsimd tree scratch
            b7scr = singles.tile([128, 27 * SLAB], F16)  # b7 chunk scratch
            # sraw: spatial sums, natural layout: [p=(4n+c_hi), c_low], fp16
            sraw_all = singles.tile([128, 128 * BPC], F16)
            # state^T: [p=c_low, 128b + 4n + c_hi], fp16
            st_all = singles.tile([128, 128 * BPC], F16)
            outsb = singles.tile([4, 128 * BPC], F32)

            a_sb = wsb[:, 0:2048]
            uv_sb = wsb[:, 2048:2056]
            msk_sb = wsb[:, 2058:2094]  # [:, :32]=M32, [:, 32:36]=mask4

            def load_weights():
                # one blob DMA on the scalar HWDGE ring so the sync ring
                # carries only the x stream
                nc.scalar.dma_start(out=wsb[:], in_=w_d[:])
                nc.scalar.copy(c0_sb[:], wsb[0:32, 2056:2057])
                masks.make_identity(nc, ident[:])
                nc.gpsimd.memset(ones16[:], 1.0)
                nc.gpsimd.memset(ones_col[:], 1.0)
                nc.gpsimd.memset(ones_r32[:], 1.0)

            # per-group state (allocated lazily in program order)
            grp = {}

            def start_group(g):
                gs = GSIZE[g]
                grp[g] = {
                    # adj [:32, 0:128]; sv rows [0:1, 128+32bp : 160+32bp]
                    "ps_adj": ps_adj_pool.tile([32, 256], F32, name="ps_adj"),
                    # ps_misc regions: su cols [:32, 0:4], stats colsum
                    # [:1, 8:16], stats bcast [:32, 16:24], wf colsums
                    # [:1, 64+32bp], wfb bcast [:, 192+32bp]
                    "ps_misc": ps_misc_pool.tile([128, 512], F32, name="ps_misc"),
                    "ps_out": ps_out_pool.tile([4, 128 * gs], F32, name="ps_out"),
                    "sv": small.tile([1, 32 * gs], F16, tag="sv", name="sv"),
                    "q": small.tile([32, gs], F32, tag="q", name="q"),
                    "t": small.tile([32, gs], F32, tag="t", name="t"),
                    "rq": small.tile([32, gs], F32, tag="rq", name="rq"),
                    "expt": small.tile([32, 32 * gs], F32, tag="expt", name="expt"),
                }

            def dma_batch(b):
                nc.sync.dma_start(
                    out=x_all[:, FREE * b : FREE * (b + 1)],
                    in_=x_d[b, :, :],
                )

            def dma_b7_chunks():
                b = BPC - 1
                s0 = 0
                for ci, nslab in enumerate(CHUNKS):
                    nc.sync.dma_start(
                        out=x_all[
                            :,
                            FREE * b + SLAB * s0 : FREE * b + SLAB * (s0 + nslab),
                        ],
                        in_=x_d[b, :, SLAB * s0 : SLAB * (s0 + nslab)],
                    )
                    s0 += nslab

            def tree_full(eng, b, scratch):
                # 49 spatial slabs -> sraw column, contiguous fp16 runs only
                xb = x_all[:, FREE * b : FREE * (b + 1)]
                t = scratch
                dst = sraw_all[:, 128 * b : 128 * (b + 1)]
                eng.tensor_add(t[:, 0:3072], xb[:, 0:3072], xb[:, 3200:6272])
                eng.tensor_add(t[:, 0:1536], t[:, 0:1536], t[:, 1536:3072])
                eng.tensor_add(t[:, 0:768], t[:, 0:768], t[:, 768:1536])
                eng.tensor_add(t[:, 0:384], t[:, 0:384], t[:, 384:768])
                eng.tensor_add(dst, t[:, 0:128], t[:, 128:256])
                eng.tensor_add(dst, dst, t[:, 256:384])
                eng.tensor_add(dst, dst, xb[:, 3072:3200])  # s=24 slab

            def tree_b7(which):
                # tapered chunks: A=24 slabs, B=16, C=8, D=1; partials land in
                # b7scr: tA 0:1536, tB 1536:2560, tC 2560:3072,
                # accA 3072:3200, accB 3200:3328, accC 3328:3456
                b = BPC - 1
                xb = x_all[:, FREE * b : FREE * (b + 1)]
                t = b7scr
                v = nc.vector
                if which == 0:
                    v.tensor_add(t[:, 0:1536], xb[:, 0:1536], xb[:, 1536:3072])
                    v.tensor_add(t[:, 0:768], t[:, 0:768], t[:, 768:1536])
                    v.tensor_add(t[:, 0:384], t[:, 0:384], t[:, 384:768])
                    v.tensor_add(t[:, 3072:3200], t[:, 0:128], t[:, 128:256])
                    v.tensor_add(t[:, 3072:3200], t[:, 3072:3200], t[:, 256:384])
                elif which == 1:
                    v.tensor_add(t[:, 1536:2560], xb[:, 3072:4096], xb[:, 4096:5120])
                    v.tensor_add(t[:, 1536:2048], t[:, 1536:2048], t[:, 2048:2560])
                    v.tensor_add(t[:, 1536:1792], t[:, 1536:1792], t[:, 1792:2048])
                    v.tensor_add(t[:, 3200:3328], t[:, 1536:1664], t[:, 1664:1792])
                elif which == 2:
                    v.tensor_add(t[:, 2560:3072], xb[:, 5120:5632], xb[:, 5632:6144])
                    v.tensor_add(t[:, 2560:2816], t[:, 2560:2816], t[:, 2816:3072])
                    v.tensor_add(t[:, 3328:3456], t[:, 2560:2688], t[:, 2688:2816])
                else:
                    dst = sraw_all[:, 128 * b : 128 * (b + 1)]
                    v.tensor_add(dst, t[:, 3072:3200], t[:, 3200:3328])
                    v.tensor_add(dst, dst, t[:, 3328:3456])
                    v.tensor_add(dst, dst, xb[:, 6144:6272])  # D slab s=48

            def st_slice(b, r):
                return st_all[:, 128 * b + r : 128 * (b + 1) : 4]

            def chain(b):
                # PE + Scalar only: transpose, TA, su/sv, adj, stats pieces.
                # No DVE ops -> tree stream never head-of-line blocks.
                g, bp = GRP_OF_B[b], BP_OF_B[b]
                gd = grp[g]
                scol = slice(128 * b, 128 * (b + 1))

                ps_t = ps_t_pool.tile([128, 128], F16)
                nc.tensor.transpose(ps_t[:], sraw_all[:, scol], ident[:])
                nc.scalar.copy(st_all[:, scol], ps_t[:])

                # -- TA^T = A^T S^T directly: [p=d_low, 32s+k] ---------------
                ps_tt = ps_tt_pool.tile([128, 128], F32)
                for s in range(4):
                    for r in range(4):
                        nc.tensor.matmul(
                            ps_tt[:, 32 * s : 32 * (s + 1)],
                            a_sb[:, 512 * r + 128 * s : 512 * r + 128 * (s + 1)],
                            st_slice(b, r),
                            start=(r == 0), stop=(r == 3),
                        )
                ta_b = tap.tile([128, 128], F16, tag="ta")
                nc.scalar.copy(ta_b[:], ps_tt[:])

                # -- su column + sv row --------------------------------------
                ps_misc = gd["ps_misc"]
                for r in range(4):
                    nc.tensor.matmul(
                        ps_misc[:32, bp : bp + 1],
                        st_slice(b, r),
                        uv_sb[:, 2 * r : 2 * r + 1],
                        start=(r == 0), stop=(r == 3),
                    )
                svsl = slice(128 + 32 * bp, 128 + 32 * (bp + 1))
                for r in range(4):
                    nc.tensor.matmul(
                        gd["ps_adj"][0:1, svsl],
                        uv_sb[:, 2 * r + 1 : 2 * r + 2],
                        st_slice(b, r),
                        start=(r == 0), stop=(r == 3),
                    )
                nc.scalar.copy(
                    gd["sv"][:, 32 * bp : 32 * (bp + 1)], gd["ps_adj"][0:1, svsl]
                )

                # -- adjacency': TA S^T + 1 sv^T (minus row-constants) -------
                ps_adj = gd["ps_adj"]
                asl = slice(32 * bp, 32 * (bp + 1))
                for s in range(4):
                    nc.tensor.matmul(
                        ps_adj[:, asl],
                        ta_b[:, 32 * s : 32 * (s + 1)],
                        st_slice(b, s),
                        start=(s == 0), stop=False,
                    )
                nc.tensor.matmul(
                    ps_adj[:, asl],
                    ones16[0:1, 0:32],
                    gd["sv"][0:1, 32 * bp : 32 * (bp + 1)],
                    start=False, stop=True,
                )

                # -- per-batch stats pieces (Scalar engine) ------------------
                id_scr = small.tile([32, 32], F32, tag="id_scr", name="id_scr")
                nc.scalar.activation(
                    out=id_scr[:], in_=ps_adj[:, asl],
                    func=mybir.ActivationFunctionType.Identity,
                    accum_out=gd["t"][:, bp : bp + 1],
                )
                sq_scr = small.tile([32, 32], F32, tag="sq_scr", name="sq_scr")
                nc.scalar.activation(
                    out=sq_scr[:], in_=ps_adj[:, asl],
                    func=mybir.ActivationFunctionType.Square,
                    accum_out=gd["rq"][:, bp : bp + 1],
                )
                nc.scalar.activation(
                    out=gd["q"][:, bp : bp + 1], in_=ps_misc[:32, bp : bp + 1],
                    func=mybir.ActivationFunctionType.Identity,
                    bias=c0_sb[:], scale=1.0,
                )

            def finish_group(g):
                gd = grp[g]
                gs = GSIZE[g]
                ps_adj, ps_misc, ps_out = gd["ps_adj"], gd["ps_misc"], gd["ps_out"]
                q_g, t_g, rowsq = gd["q"], gd["t"], gd["rq"]

                # ---- stats: S1/S2 of TRUE adj' via row sums ----------------
                # stats_g: cols 0:gs = S1 rows, gs:2gs = S2 rows
                stats_g = small.tile([32, 2 * gs], F32, tag="stats_g", name="sg")
                q32 = small.tile([32, gs], F32, tag="q32", name="q32")
                nc.vector.tensor_scalar(
                    out=q32[:], in0=q_g[:], scalar1=32.0, scalar2=None, op0=A_.mult,
                )
                nc.vector.tensor_add(stats_g[:, 0:gs], q32[:], t_g[:])
                # S2row = rowsq + q*(2t + 32q); 2t + 32q = t + S1row
                h_g = small.tile([32, gs], F32, tag="h_g", name="h_g")
                nc.vector.tensor_add(h_g[:], t_g[:], stats_g[:, 0:gs])
                s2c = small.tile([32, gs], F32, tag="s2c", name="s2c")
                nc.vector.tensor_mul(s2c[:], q_g[:], h_g[:])
                nc.vector.tensor_add(stats_g[:, gs : 2 * gs], rowsq[:], s2c[:])

                # cross-partition sum + broadcast back (PE ones trick)
                nc.tensor.matmul(
                    ps_misc[:1, 8 : 8 + 2 * gs], ones_col[:], stats_g[:],
                    start=True, stop=True,
                )
                s_sb = small.tile([1, 2 * gs], F32, tag="s_sb", name="s_sb")
                nc.scalar.copy(s_sb[:], ps_misc[:1, 8 : 8 + 2 * gs])
                nc.tensor.matmul(
                    ps_misc[:32, 16 : 16 + 2 * gs], ones_r32[:], s_sb[:],
                    start=True, stop=True,
                )
                s_all = small.tile([32, 2 * gs], F32, tag="s_all", name="s_all")
                nc.scalar.copy(s_all[:], ps_misc[:32, 16 : 16 + 2 * gs])

                # ---- inv_std = sqrt(1023)/sqrt(S2 - S1^2/1024) -------------
                t1 = small.tile([32, gs], F32, tag="t1", name="t1")
                nc.vector.tensor_mul(t1[:], s_all[:, 0:gs], s_all[:, 0:gs])
                nc.vector.tensor_scalar(
                    out=t1[:], in0=t1[:], scalar1=-1.0 / 1024.0, scalar2=None,
                    op0=A_.mult,
                )
                v1023 = small.tile([32, gs], F32, tag="v1023", name="v1023")
                nc.vector.tensor_add(v1023[:], t1[:], s_all[:, gs : 2 * gs])
                # Newton rsqrt with magic seed, 2 iterations (~4e-6 rel err);
                # exp stays the only activation table the scalar engine needs
                yint = small.tile([32, gs], I32, tag="yint", name="yint")
                nc.vector.tensor_scalar(
                    out=yint[:], in0=v1023[:].bitcast(I32), scalar1=1,
                    scalar2=None, op0=A_.logical_shift_right,
                )
                nc.vector.tensor_scalar(
                    out=yint[:], in0=yint[:], scalar1=-1,
                    scalar2=0x5F3759DF, op0=A_.mult, op1=A_.add,
                )
                y = small.tile([32, gs], F32, tag="y", name="y")
                nc.vector.tensor_copy(y[:], yint[:].bitcast(F32))
                ya = small.tile([32, gs], F32, tag="ya", name="ya")
                yb = small.tile([32, gs], F32, tag="yb", name="yb")
                for it in range(2):
                    nc.vector.tensor_mul(ya[:], y[:], y[:])
                    nc.vector.tensor_mul(yb[:], ya[:], v1023[:])
                    last = it == 1
                    nc.vector.tensor_scalar(
                        out=ya[:], in0=yb[:],
                        scalar1=(-0.5 * K1023) if last else -0.5,
                        scalar2=(1.5 * K1023) if last else 1.5,
                        op0=A_.mult, op1=A_.add,
                    )
                    nc.vector.tensor_mul(y[:], y[:], ya[:])
                inv_g = y  # [32, gs] inv_std per batch column

                # ---- softmax + epilogue ------------------------------------
                # no row-max subtraction: after global mean/std normalization
                # the exp argument is bounded (|z| < ~10), safe in f32; row
                # constants cancel in the row softmax
                expt = gd["expt"]
                for bp in range(gs):
                    nc.scalar.activation(
                        out=expt[:, 32 * bp : 32 * (bp + 1)],
                        in_=ps_adj[:, 32 * bp : 32 * (bp + 1)],
                        func=mybir.ActivationFunctionType.Exp,
                        bias=0.0, scale=inv_g[:, bp : bp + 1],
                    )
                rowsum = small.tile([32, gs], F32, tag="rowsum", name="rs")
                nc.vector.reduce_sum(
                    out=rowsum[:],
                    in_=expt[:, 0 : 32 * gs].rearrange("p (b m) -> p b m", m=32),
                    axis=mybir.AxisListType.X,
                )
                recip = small.tile([32, gs], F32, tag="recip", name="recip")
                nc.vector.reciprocal(recip[:], rowsum[:])

                # w[k] = colsum(soft) per batch -> [1, 32] rows at partition 0
                for bp in range(gs):
                    nc.tensor.matmul(
                        ps_misc[:1, 64 + 32 * bp : 64 + 32 * (bp + 1)],
                        recip[:, bp : bp + 1],
                        expt[:, 32 * bp : 32 * (bp + 1)],
                        start=True, stop=True,
                    )
                wf16 = small.tile([1, 32 * gs], F16, tag="wf16", name="wf16")
                nc.vector.tensor_scalar(
                    out=wf16[:], in0=ps_misc[:1, 64 : 64 + 32 * gs],
                    scalar1=1.0 / (N * HW), scalar2=1.0 / (N * HW),
                    op0=A_.mult, op1=A_.add,
                )
                for bp in range(gs):
                    b = GBASE[g] + bp
                    # broadcast wf to all partitions, then gather the diagonal
                    # wfb[p, p//4] and spread to wf4[p, r] = w[p//4]*(p%4==r)
                    wsl = slice(192 + 32 * bp, 224 + 32 * bp)
                    nc.tensor.matmul(
                        ps_misc[:, wsl],
                        ones16[:],
                        wf16[0:1, 32 * bp : 32 * (bp + 1)],
                        start=True, stop=True,
                    )
                    scr32 = small.tile([128, 32], F32, tag="scr32", name="scr32")
                    dcol = small.tile([128, 1], F32, tag="dcol", name="dcol")
                    nc.vector.scalar_tensor_tensor(
                        out=scr32[:], in0=ps_misc[:, wsl], scalar=1.0,
                        in1=msk_sb[:, 0:32], op0=A_.mult, op1=A_.mult,
                        accum_out=dcol[:],
                    )
                    wf4 = small.tile([128, 4], F16, tag="wf4", name="wf4")
                    nc.gpsimd.tensor_scalar(
                        out=wf4[:], in0=msk_sb[:, 32:36], scalar1=dcol[:],
                        scalar2=None, op0=A_.mult,
                    )
                    nc.tensor.matmul(
                        ps_out[:4, 128 * bp : 128 * (bp + 1)],
                        wf4[:],
                        sraw_all[:, 128 * b : 128 * (b + 1)],
                        start=True, stop=True,
                    )
                    nc.scalar.copy(
                        outsb[:, 128 * b : 128 * (b + 1)],
                        ps_out[:4, 128 * bp : 128 * (bp + 1)],
                    )
                gb = GBASE[g]
                nc.sync.dma_start(
                    out=out_d[:, 128 * gb : 128 * (gb + gs)],
                    in_=outsb[:, 128 * gb : 128 * (gb + gs)],
                )

            # ---- schedule --------------------------------------------------
            # All x DMAs first on the sync ring (stream never waits), weights
            # on the scalar ring.  GpSimd takes batch 0's tree; DVE does the
            # rest.  chain() has no DVE ops, so the DVE queue is trees +
            # finish chains only, ordered to match data arrival.
            for b in range(BPC - 1):
                dma_batch(b)
            dma_b7_chunks()
            load_weights()

            start_group(0)
            tree_full(nc.gpsimd, 0, gscr)          # b0 on GpSimd
            for b in (1, 2, 3):
                sc = tgpool.tile([128, 3072], F16, tag="tg", name="tg")
                tree_full(nc.vector, b, sc)
                chain(b)
            chain(0)
            start_group(1)
            sc = tgpool.tile([128, 3072], F16, tag="tg", name="tg")
            tree_full(nc.vector, 4, sc)
            chain(4)
            finish_group(0)
            for b in (5, 6):
                sc = tgpool.tile([128, 3072], F16, tag="tg", name="tg")
                tree_full(nc.vector, b, sc)
                chain(b)
            start_group(2)
            for ci in range(4):
                tree_b7(ci)
            chain(7)
            finish_group(1)
            finish_group(2)

    lp.__exit__(None, None, None)
    nc.finalize()
    return nc


def host_prep(input, W1, b1, W2, b2):
    # x: [B, N, C, 7, 7] f32 -> fp16, partition p = 4n + c_hi, per-partition
    # row layout [s(49), c_low(128)]
    x = np.asarray(input, dtype=np.float32)
    xt = (
        x.reshape(B, N, 4, 128, HW)
        .transpose(0, 1, 2, 4, 3)
        .astype(np.float16)
        .reshape(B, 128, FREE)
    )
    w1 = np.asarray(W1, dtype=np.float64)
    w2 = np.asarray(W2, dtype=np.float64)
    b1 = np.asarray(b1, dtype=np.float64)
    b2 = np.asarray(b2, dtype=np.float64)
    # softmax((adj-mu)/sigma) is scale-invariant per batch: use 2401*adj so A
    # stays in fp16-normal range
    amat = (w1.T @ w2).astype(np.float16)
    u = HW * (w1.T @ b2)
    v = HW * (w2.T @ b1)
    uv = np.stack([u, v], axis=1).astype(np.float16)
    c0 = float(HW * HW * (b1 @ b2))
    p = np.arange(128)
    m32 = (np.arange(32)[None, :] == (p[:, None] // 4)).astype(np.float16)
    m4 = (np.arange(4)[None, :] == (p[:, None] % 4)).astype(np.float16)

    wblob = np.zeros((128, WCOLS), dtype=np.float16)
    wblob[:, 0:2048] = amat.reshape(4, 128, 512).transpose(1, 0, 2).reshape(128, 2048)
    wblob[:, 2048:2056] = uv.reshape(4, 128, 2).transpose(1, 0, 2).reshape(128, 8)
    wblob[:, 2056] = c0
    wblob[:, 2058:2090] = m32
    wblob[:, 2090:2094] = m4
    return xt, np.ascontiguousarray(wblob)


def make_in_maps(input, W1, b1, W2, b2):
    xt, wblob = host_prep(input, W1, b1, W2, b2)
    in_maps = []
    for i in range(NCORES):
        shard = np.ascontiguousarray(xt[BPC * i : BPC * (i + 1)])
        in_maps.append({"x": shard, "wblob": wblob})
    return in_maps


def kernel(input, W1, b1, W2, b2):
    global _CACHED_NC
    if _CACHED_NC is None:
        _CACHED_NC = build_bass()
    nc = _CACHED_NC

    in_maps = make_in_maps(input, W1, b1, W2, b2)
    res = run_bass_kernel_spmd(nc, in_maps, list(range(NCORES)))

    out = np.empty((B, C), dtype=np.float32)
    for i in range(NCORES):
        o = res.results[i]["out"]  # [4, 128*BPC], out[b, 128r+q] = o[r, 128b+q]
        out[BPC * i : BPC * (i + 1)] = (
            o.reshape(4, BPC, 128).transpose(1, 0, 2).reshape(BPC, C)
        )
    return out


# revision 12
# speedup vs baseline: 1.4460x; 1.2501x over previous
"""Trainium2 Bass kernel for nn_ConvGraph_SC (gnn_message_passing).

Reference computation (per batch b of 64, N=32 nodes, C=512 channels, 7x7 spatial):
    state = input.mean(axis=(3,4))                       # [B, N, C]
    mat1  = state @ W1.T + b1
    mat2  = state @ W2.T + b2
    adj   = mat1 @ mat2.T                                # [B, N, N]
    soft  = softmax((adj - mean(adj)) / std(adj), rows)  # global mean/std, ddof=1
    out   = mean(soft @ state + state, axis=1)           # [B, C]

Device-side algebra (S = spatial SUM of x, unscaled):
  * softmax((adj-mu)/sigma) is invariant to a per-batch affine rescale of adj,
    so we work with adj' = 2401*adj = S A S^T + 49*s_u 1^T + 49*1 s_v^T + c0'
    where A = W1^T W2, u' = 49 W1^T b2, v' = 49 W2^T b1, c0' = 2401 b1.b2.
  * Row-constant terms (s_u, c0) drop out of the row softmax; they enter only
    the global mean/std, computed from per-row sums with closed-form
    corrections. No row-max subtraction before exp: after global mean/std
    normalization the argument is bounded (|z| ~< 10), safe in f32.
  * x is shipped fp16 (rel-err budget is 2e-2; fp16 rounding of the spatial
    sum lands ~5e-4) -> HBM traffic halves to ~13.1 MB/core + 0.54 MB weights.
  * Per-partition row layout is [s(49), c_low(128)] so the spatial-sum
    pairwise-add tree works on contiguous, 4B-aligned fp16 runs -> DVE
    tensor_tensor 2x mode (2 elem/cycle), 3 in-place ops per batch folding
    49 slabs to 6 partials + the s24 leftover; the PE consumes those 7
    pieces directly (transpose and epilogue are linear in S, so the summed
    sraw never materializes and the small tree levels never run on DVE).
  * All 8 batch buffers stay SBUF-resident (~100 KB/partition) so the input
    stream never stalls on buffer recycling.
  * Per-batch adj row sums / row sums-of-squares run on the Scalar engine
    (activation accum_out), keeping DVE free for the trees; softmax row
    sums fall out of the Exp activation's accum_out for free.
  * Epilogue out[c] = sum_k (colsum(soft)[k]+1)/(N*HW) * S[k,c] runs on the
    PE: stationary wf4[p,r] = w[p//4]*(p%4==r) against the moving pieces.

Sharding: pure data parallel, 8 batches per NeuronCore, weights replicated.
"""

import numpy as np

import concourse.bacc as bacc
import concourse.tile as tile
from concourse import masks, mybir
from concourse.bass_utils import run_bass_kernel_spmd

F32 = mybir.dt.float32
F16 = mybir.dt.float16
I32 = mybir.dt.int32
NCORES = 8
B, N, C, HW = 64, 32, 512, 49
BPC = B // NCORES          # batches per core
FREE = N * C * HW // 128   # 6272 fp16 elems per partition per batch
SLAB = 128                 # one spatial position = 128 c_low elems
K1023 = float(np.sqrt(np.float64(1023.0)))
# stats groups [4, 3, 1]: the tail after the last input byte is only batch
# 7's single-batch chain
GSIZE = [4, 3, 1]
GBASE = [0, 4, 7]
GRP_OF_B = [0, 0, 0, 0, 1, 1, 1, 2]
BP_OF_B = [0, 1, 2, 3, 0, 1, 2, 0]
# weights blob columns (fp16): amat 0:2048, uv 2048:2056, c0 2056, msk 2058:2094
WCOLS = 2096

_CACHED_NC = None

A_ = mybir.AluOpType


def build_bass():
    nc = bacc.Bacc("TRN2", target_bir_lowering=False)

    x_d = nc.declare_dram_parameter("x", [BPC, 128, FREE], F16, isOutput=False)
    w_d = nc.declare_dram_parameter("wblob", [128, WCOLS], F16, isOutput=False)
    out_d = nc.declare_dram_parameter("out", [4, 128 * BPC], F32, isOutput=True)

    lp = nc.allow_low_precision("fp16 input + spatial sums; rel-err budget 2e-2")
    lp.__enter__()
    with tile.TileContext(nc) as tc:
        with (
            tc.tile_pool(name="singles", bufs=1) as singles,
            tc.tile_pool(name="tap", bufs=2) as tap,
            tc.tile_pool(name="small", bufs=2) as small,
            tc.tile_pool(name="ps_t", bufs=1, space="PSUM") as ps_t_pool,
            tc.tile_pool(name="ps_tt", bufs=2, space="PSUM") as ps_tt_pool,
            tc.tile_pool(name="ps_adj", bufs=2, space="PSUM") as ps_adj_pool,
            tc.tile_pool(name="ps_misc", bufs=2, space="PSUM") as ps_misc_pool,
            tc.tile_pool(name="ps_out", bufs=1, space="PSUM") as ps_out_pool,
        ):
            # ---- persistent tiles -----------------------------------------
            x_all = singles.tile([128, FREE * BPC], F16)
            ident = singles.tile([128, 128], F16)
            ones16 = singles.tile([1, 128], F16)
            ones_col = singles.tile([32, 1], F32)
            ones_r32 = singles.tile([1, 32], F32)
            wsb = singles.tile([128, WCOLS], F16)
            c0_sb = singles.tile([32, 1], F32)
            wnorm = singles.tile([1, 1], F32)  # 1/(N*HW) for the wf16 scaling
            # state^T: [p=c_low, 128b + 4n + c_hi], fp16
            st_all = singles.tile([128, 128 * BPC], F16)
            outsb = singles.tile([4, 128 * BPC], F32)

            a_sb = wsb[:, 0:2048]
            uv_sb = wsb[:, 2048:2056]
            msk_sb = wsb[:, 2058:2094]  # [:, :32]=M32, [:, 32:36]=mask4

            def load_weights():
                # one blob DMA on the scalar HWDGE ring so the sync ring
                # carries only the x stream
                nc.scalar.dma_start(out=wsb[:], in_=w_d[:])
                nc.scalar.copy(c0_sb[:], wsb[0:32, 2056:2057])
                masks.make_identity(nc, ident[:])
                nc.gpsimd.memset(ones16[:], 1.0)
                nc.gpsimd.memset(ones_col[:], 1.0)
                nc.gpsimd.memset(ones_r32[:], 1.0)
                nc.gpsimd.memset(wnorm[:], 1.0 / (N * HW))

            # per-group state (allocated lazily in program order)
            grp = {}

            def start_group(g):
                gs = GSIZE[g]
                grp[g] = {
                    # adj [:32, 0:128]; sv rows [0:1, 128+32bp : 160+32bp]
                    "ps_adj": ps_adj_pool.tile([32, 256], F32, name="ps_adj"),
                    # ps_misc regions: su cols [:32, 0:4], stats colsum
                    # [:1, 8:16], stats bcast [:32, 16:24], wf colsums
                    # [:1, 64+32bp], wfb bcast [:, 192+32bp]
                    "ps_misc": ps_misc_pool.tile([128, 512], F32, name="ps_misc"),
                    "ps_out": ps_out_pool.tile([4, 128 * gs], F32, name="ps_out"),
                    "sv": small.tile([1, 32 * gs], F16, tag="sv", name="sv"),
                    "q": small.tile([32, gs], F32, tag="q", name="q"),
                    "t": small.tile([32, gs], F32, tag="t", name="t"),
                    "rq": small.tile([32, gs], F32, tag="rq", name="rq"),
                    "expt": small.tile([32, 32 * gs], F32, tag="expt", name="expt"),
                }

            def dma_batch(b):
                nc.sync.dma_start(
                    out=x_all[:, FREE * b : FREE * (b + 1)], in_=x_d[b, :, :]
                )

            def tree(b):
                # fold 49 slabs to 6 partial slabs fully in place (2x-mode
                # contiguous fp16 adds); s24 is left untouched and the PE
                # consumes the 7 pieces directly
                xb = x_all[:, FREE * b : FREE * (b + 1)]
                v = nc.vector
                v.tensor_add(xb[:, 0:3072], xb[:, 0:3072], xb[:, 3200:6272])
                v.tensor_add(xb[:, 0:1536], xb[:, 0:1536], xb[:, 1536:3072])
                v.tensor_add(xb[:, 0:768], xb[:, 0:768], xb[:, 768:1536])

            def pieces(b):
                xb = x_all[:, FREE * b : FREE * (b + 1)]
                return [xb[:, 128 * k : 128 * (k + 1)] for k in range(6)] + [
                    xb[:, 3072:3200]
                ]

            def st_slice(b, r):
                return st_all[:, 128 * b + r : 128 * (b + 1) : 4]

            def chain(b):
                # PE + Scalar only: transpose, TA, su/sv, adj, stats pieces.
                # No DVE ops -> tree stream never head-of-line blocks.
                g, bp = GRP_OF_B[b], BP_OF_B[b]
                gd = grp[g]
                scol = slice(128 * b, 128 * (b + 1))

                # st = sum of piece transposes (transpose is linear in S)
                ps_t = ps_t_pool.tile([128, 128], F32)
                pcs = pieces(b)
                for k, pc in enumerate(pcs):
                    nc.tensor.matmul(
                        ps_t[:], pc, ident[:],
                        start=(k == 0), stop=(k == len(pcs) - 1),
                    )
                nc.scalar.copy(st_all[:, scol], ps_t[:])

                # -- TA^T = A^T S^T directly: [p=d_low, 32s+k] ---------------
                ps_tt = ps_tt_pool.tile([128, 128], F32)
                for s in range(4):
                    for r in range(4):
                        nc.tensor.matmul(
                            ps_tt[:, 32 * s : 32 * (s + 1)],
                            a_sb[:, 512 * r + 128 * s : 512 * r + 128 * (s + 1)],
                            st_slice(b, r),
                            start=(r == 0), stop=(r == 3),
                        )
                ta_b = tap.tile([128, 128], F16, tag="ta")
                nc.scalar.copy(ta_b[:], ps_tt[:])

                # -- su column + sv row --------------------------------------
                ps_misc = gd["ps_misc"]
                for r in range(4):
                    nc.tensor.matmul(
                        ps_misc[:32, bp : bp + 1],
                        st_slice(b, r),
                        uv_sb[:, 2 * r : 2 * r + 1],
                        start=(r == 0), stop=(r == 3),
                    )
                svsl = slice(128 + 32 * bp, 128 + 32 * (bp + 1))
                for r in range(4):
                    nc.tensor.matmul(
                        gd["ps_adj"][0:1, svsl],
                        uv_sb[:, 2 * r + 1 : 2 * r + 2],
                        st_slice(b, r),
                        start=(r == 0), stop=(r == 3),
                    )
                nc.scalar.copy(
                    gd["sv"][:, 32 * bp : 32 * (bp + 1)], gd["ps_adj"][0:1, svsl]
                )

                # -- adjacency': TA S^T + 1 sv^T (minus row-constants) -------
                ps_adj = gd["ps_adj"]
                asl = slice(32 * bp, 32 * (bp + 1))
                for s in range(4):
                    nc.tensor.matmul(
                        ps_adj[:, asl],
                        ta_b[:, 32 * s : 32 * (s + 1)],
                        st_slice(b, s),
                        start=(s == 0), stop=False,
                    )
                nc.tensor.matmul(
                    ps_adj[:, asl],
                    ones16[0:1, 0:32],
                    gd["sv"][0:1, 32 * bp : 32 * (bp + 1)],
                    start=False, stop=True,
                )

                # -- per-batch stats pieces (Scalar engine) ------------------
                id_scr = small.tile([32, 32], F32, tag="id_scr", name="id_scr")
                nc.scalar.activation(
                    out=id_scr[:], in_=ps_adj[:, asl],
                    func=mybir.ActivationFunctionType.Identity,
                    accum_out=gd["t"][:, bp : bp + 1],
                )
                sq_scr = small.tile([32, 32], F32, tag="sq_scr", name="sq_scr")
                nc.scalar.activation(
                    out=sq_scr[:], in_=ps_adj[:, asl],
                    func=mybir.ActivationFunctionType.Square,
                    accum_out=gd["rq"][:, bp : bp + 1],
                )
                nc.scalar.activation(
                    out=gd["q"][:, bp : bp + 1], in_=ps_misc[:32, bp : bp + 1],
                    func=mybir.ActivationFunctionType.Identity,
                    bias=c0_sb[:], scale=1.0,
                )

            def finish_group(g):
                gd = grp[g]
                gs = GSIZE[g]
                ps_adj, ps_misc, ps_out = gd["ps_adj"], gd["ps_misc"], gd["ps_out"]
                q_g, t_g, rowsq = gd["q"], gd["t"], gd["rq"]

                # ---- stats: S1/S2 of TRUE adj' via row sums ----------------
                # stats_g: cols 0:gs = S1 rows, gs:2gs = S2 rows
                stats_g = small.tile([32, 2 * gs], F32, tag="stats_g", name="sg")
                q32 = small.tile([32, gs], F32, tag="q32", name="q32")
                nc.vector.tensor_scalar(
                    out=q32[:], in0=q_g[:], scalar1=32.0, scalar2=None, op0=A_.mult,
                )
                nc.vector.tensor_add(stats_g[:, 0:gs], q32[:], t_g[:])
                # S2row = rowsq + q*(2t + 32q); 2t + 32q = t + S1row
                h_g = small.tile([32, gs], F32, tag="h_g", name="h_g")
                nc.vector.tensor_add(h_g[:], t_g[:], stats_g[:, 0:gs])
                s2c = small.tile([32, gs], F32, tag="s2c", name="s2c")
                nc.vector.tensor_mul(s2c[:], q_g[:], h_g[:])
                nc.vector.tensor_add(stats_g[:, gs : 2 * gs], rowsq[:], s2c[:])

                # cross-partition sum + broadcast back (PE ones trick)
                nc.tensor.matmul(
                    ps_misc[:1, 8 : 8 + 2 * gs], ones_col[:], stats_g[:],
                    start=True, stop=True,
                )
                s_sb = small.tile([1, 2 * gs], F32, tag="s_sb", name="s_sb")
                nc.scalar.copy(s_sb[:], ps_misc[:1, 8 : 8 + 2 * gs])
                nc.tensor.matmul(
                    ps_misc[:32, 16 : 16 + 2 * gs], ones_r32[:], s_sb[:],
                    start=True, stop=True,
                )
                s_all = small.tile([32, 2 * gs], F32, tag="s_all", name="s_all")
                nc.scalar.copy(s_all[:], ps_misc[:32, 16 : 16 + 2 * gs])

                # ---- inv_std = sqrt(1023)/sqrt(S2 - S1^2/1024) -------------
                t1 = small.tile([32, gs], F32, tag="t1", name="t1")
                nc.vector.tensor_mul(t1[:], s_all[:, 0:gs], s_all[:, 0:gs])
                nc.vector.tensor_scalar(
                    out=t1[:], in0=t1[:], scalar1=-1.0 / 1024.0, scalar2=None,
                    op0=A_.mult,
                )
                v1023 = small.tile([32, gs], F32, tag="v1023", name="v1023")
                nc.vector.tensor_add(v1023[:], t1[:], s_all[:, gs : 2 * gs])
                # Newton rsqrt with magic seed, 2 iterations (~4e-6 rel err);
                # exp stays the only activation table the scalar engine needs
                yint = small.tile([32, gs], I32, tag="yint", name="yint")
                nc.vector.tensor_scalar(
                    out=yint[:], in0=v1023[:].bitcast(I32), scalar1=1,
                    scalar2=None, op0=A_.logical_shift_right,
                )
                nc.vector.tensor_scalar(
                    out=yint[:], in0=yint[:], scalar1=-1,
                    scalar2=0x5F3759DF, op0=A_.mult, op1=A_.add,
                )
                y = small.tile([32, gs], F32, tag="y", name="y")
                nc.vector.tensor_copy(y[:], yint[:].bitcast(F32))
                ya = small.tile([32, gs], F32, tag="ya", name="ya")
                yb = small.tile([32, gs], F32, tag="yb", name="yb")
                for it in range(2):
                    nc.vector.tensor_mul(ya[:], y[:], y[:])
                    nc.vector.tensor_mul(yb[:], ya[:], v1023[:])
                    last = it == 1
                    nc.vector.tensor_scalar(
                        out=ya[:], in0=yb[:],
                        scalar1=(-0.5 * K1023) if last else -0.5,
                        scalar2=(1.5 * K1023) if last else 1.5,
                        op0=A_.mult, op1=A_.add,
                    )
                    nc.vector.tensor_mul(y[:], y[:], ya[:])
                inv_g = y  # [32, gs] inv_std per batch column

                # ---- softmax + epilogue ------------------------------------
                # no row-max subtraction: after global mean/std normalization
                # the exp argument is bounded (|z| < ~10), safe in f32; row
                # constants cancel in the row softmax
                expt = gd["expt"]
                rowsum = small.tile([32, gs], F32, tag="rowsum", name="rs")
                for bp in range(gs):
                    nc.scalar.activation(
                        out=expt[:, 32 * bp : 32 * (bp + 1)],
                        in_=ps_adj[:, 32 * bp : 32 * (bp + 1)],
                        func=mybir.ActivationFunctionType.Exp,
                        bias=0.0, scale=inv_g[:, bp : bp + 1],
                        accum_out=rowsum[:, bp : bp + 1],
                    )
                recip = small.tile([32, gs], F32, tag="recip", name="recip")
                nc.vector.reciprocal(recip[:], rowsum[:])

                # w[k] = colsum(soft) per batch -> [1, 32] rows at partition 0
                for bp in range(gs):
                    nc.tensor.matmul(
                        ps_misc[:1, 64 + 32 * bp : 64 + 32 * (bp + 1)],
                        recip[:, bp : bp + 1],
                        expt[:, 32 * bp : 32 * (bp + 1)],
                        start=True, stop=True,
                    )
                wf16 = small.tile([1, 32 * gs], F16, tag="wf16", name="wf16")
                nc.scalar.activation(
                    out=wf16[:], in_=ps_misc[:1, 64 : 64 + 32 * gs],
                    func=mybir.ActivationFunctionType.Identity,
                    bias=wnorm[:], scale=wnorm[:],
                )
                for bp in range(gs):
                    b = GBASE[g] + bp
                    # broadcast wf to all partitions, then gather the diagonal
                    # wfb[p, p//4] and spread to wf4[p, r] = w[p//4]*(p%4==r)
                    wsl = slice(192 + 32 * bp, 224 + 32 * bp)
                    nc.tensor.matmul(
                        ps_misc[:, wsl],
                        ones16[:],
                        wf16[0:1, 32 * bp : 32 * (bp + 1)],
                        start=True, stop=True,
                    )
                    scr32 = small.tile([128, 32], F32, tag="scr32", name="scr32")
                    dcol = small.tile([128, 1], F32, tag="dcol", name="dcol")
                    nc.vector.scalar_tensor_tensor(
                        out=scr32[:], in0=ps_misc[:, wsl], scalar=1.0,
                        in1=msk_sb[:, 0:32], op0=A_.mult, op1=A_.mult,
                        accum_out=dcol[:],
                    )
                    wf4 = small.tile([128, 4], F16, tag="wf4", name="wf4")
                    nc.gpsimd.tensor_scalar(
                        out=wf4[:], in0=msk_sb[:, 32:36], scalar1=dcol[:],
                        scalar2=None, op0=A_.mult,
                    )
                    pcs = pieces(b)
                    for k, pc in enumerate(pcs):
                        nc.tensor.matmul(
                            ps_out[:4, 128 * bp : 128 * (bp + 1)],
                            wf4[:],
                            pc,
                            start=(k == 0), stop=(k == len(pcs) - 1),
                        )
                    nc.scalar.copy(
                        outsb[:, 128 * b : 128 * (b + 1)],
                        ps_out[:4, 128 * bp : 128 * (bp + 1)],
                    )
                gb = GBASE[g]
                nc.sync.dma_start(
                    out=out_d[:, 128 * gb : 128 * (gb + gs)],
                    in_=outsb[:, 128 * gb : 128 * (gb + gs)],
                )

            # ---- schedule --------------------------------------------------
            # load_weights first so the GpSimd setup (identity, memsets)
            # precedes the accum-DMA descgen ops in the GpSimd queue; the x
            # base stream owns the sync ring, weights ride the scalar ring.
            # chain() has no DVE ops, so the DVE queue is trees + finish
            # chains only, ordered to match data arrival.
            load_weights()
            for b in range(BPC):
                dma_batch(b)

            start_group(0)
            for b in (0, 1, 2, 3):
                tree(b)
                chain(b)
            start_group(1)
            tree(4)
            chain(4)
            tree(5)
            chain(5)
            finish_group(0)
            tree(6)
            chain(6)
            start_group(2)
            tree(7)
            chain(7)
            finish_group(1)
            finish_group(2)

    lp.__exit__(None, None, None)
    nc.finalize()
    return nc


def host_prep(input, W1, b1, W2, b2):
    # x: [B, N, C, 7, 7] f32 -> fp16, partition p = 4n + c_hi, per-partition
    # row layout [s(49), c_low(128)]
    x = np.asarray(input, dtype=np.float32)
    xt = (
        x.reshape(B, N, 4, 128, HW)
        .transpose(0, 1, 2, 4, 3)
        .astype(np.float16)
        .reshape(B, 128, FREE)
    )
    w1 = np.asarray(W1, dtype=np.float64)
    w2 = np.asarray(W2, dtype=np.float64)
    b1 = np.asarray(b1, dtype=np.float64)
    b2 = np.asarray(b2, dtype=np.float64)
    # softmax((adj-mu)/sigma) is scale-invariant per batch: use 2401*adj so A
    # stays in fp16-normal range
    amat = (w1.T @ w2).astype(np.float16)
    u = HW * (w1.T @ b2)
    v = HW * (w2.T @ b1)
    uv = np.stack([u, v], axis=1).astype(np.float16)
    c0 = float(HW * HW * (b1 @ b2))
    p = np.arange(128)
    m32 = (np.arange(32)[None, :] == (p[:, None] // 4)).astype(np.float16)
    m4 = (np.arange(4)[None, :] == (p[:, None] % 4)).astype(np.float16)

    wblob = np.zeros((128, WCOLS), dtype=np.float16)
    wblob[:, 0:2048] = amat.reshape(4, 128, 512).transpose(1, 0, 2).reshape(128, 2048)
    wblob[:, 2048:2056] = uv.reshape(4, 128, 2).transpose(1, 0, 2).reshape(128, 8)
    wblob[:, 2056] = c0
    wblob[:, 2058:2090] = m32
    wblob[:, 2090:2094] = m4
    return xt, np.ascontiguousarray(wblob)


def make_in_maps(input, W1, b1, W2, b2):
    xt, wblob = host_prep(input, W1, b1, W2, b2)
    in_maps = []
    for i in range(NCORES):
        shard = np.ascontiguousarray(xt[BPC * i : BPC * (i + 1)])
        in_maps.append({"x": shard, "wblob": wblob})
    return in_maps


def kernel(input, W1, b1, W2, b2):
    global _CACHED_NC
    if _CACHED_NC is None:
        _CACHED_NC = build_bass()
    nc = _CACHED_NC

    in_maps = make_in_maps(input, W1, b1, W2, b2)
    res = run_bass_kernel_spmd(nc, in_maps, list(range(NCORES)))

    out = np.empty((B, C), dtype=np.float32)
    for i in range(NCORES):
        o = res.results[i]["out"]  # [4, 128*BPC], out[b, 128r+q] = o[r, 128b+q]
        out[BPC * i : BPC * (i + 1)] = (
            o.reshape(4, BPC, 128).transpose(1, 0, 2).reshape(BPC, C)
        )
    return out
